# revision 14
# baseline (speedup 1.0000x reference)
"""Trainium2 Bass kernel for CAttention (contextual attention).

Math (per batch element, derived from the reference):
    x:    (c=128, h=64, w=64), flat (128, 4096); m: (1, 4096)
    k    = normalize_rows(x.reshape(c, hw).T + eps)          # (4096, 128)
    y    = 3x3 zero-padded box filter of x                   # (128, 4096)
    S    = k @ y                                             # (4096 l, 4096 ij)
    att  = softmax over l (per column); constant-shift trick: softmax needs no
           per-column max because S is bounded (|S| <= ~34, col max >= ~11):
           u = exp(S - 20), att = u / colsum(u)
    rec  = k.T @ att                                         # (128, 4096)
    out  = rec * (1-m)/9 + x*m
    (eps=1e-7 is dropped on-chip: its effect is ~1e-7 relative, far below the
     accuracy gate)

Sharding: pure data parallel over batch (4) x output-column halves (2) = 8
cores, zero cross-core communication. Each core: full l = 4096, its 2048
output columns.

v2 design (from the v1 trace: ACT exp stream = the bottleneck spine; fill was
29us, drain 17us):
  - The 64 exps (ACT, ~1.18us each, dtype-independent rate) are the hard
    floor (~76us). Everything else is arranged to fit underneath and the
    fill/drain around the stream is minimized.
  - exp outputs bf16: DVE sum-tree gets 2x throughput, SBUF traffic halves.
  - norm2 = sum_c x^2: first 4 l-tiles via ACT Square(+accum) during fill;
    rest via gpsimd x*x multiplies + DVE segmented tensor_reduce (keeps the
    ACT free for exps).
  - mm1 stationary is x itself (dram/sbuf tiles typed f32r, no eps pass).
  - software-pipelined unified 64-step loop: mm1 emitted one step ahead of
    its exp; mm2 of the second block lagged a few steps so the block-0
    epilogue (which holds the single rec PSUM buffer) never stalls block-1
    mm1s (ACT never starves).
  - mask row shipped as [1, 2048] and broadcast to 128 partitions by DMA.
"""

import numpy as np

SHIFT = 20.0
C = 128          # channels
L = 4096         # spatial locations (l axis)
HALF = 2048      # output columns per core
BLK = 1024       # ij block (psum-bank sized: 2 banks)
NLT = 32         # l tiles of 128
YW = 2176        # xyh width: 34 padded image rows x 64
NSTEP = 2 * NLT  # unified (blk, lt) steps

_CACHE = {}
DEBUG = False


def _build_program():
    import concourse.bass as bass
    import concourse.bacc as bacc
    import concourse.tile as tile
    import concourse.mybir as mybir

    F32 = mybir.dt.float32
    F32R = mybir.dt.float32r
    BF16 = mybir.dt.bfloat16
    AF = mybir.ActivationFunctionType
    ALU = mybir.AluOpType
    AX = mybir.AxisListType
    I32 = mybir.dt.int32

    nc = bacc.Bacc("TRN2", target_bir_lowering=False, num_swdge_queues=4)

    # x typed f32r end-to-end: used only as the mm1 stationary operand
    x_d = nc.dram_tensor("x", [C, L], F32R, kind="ExternalInput")
    # xt pre-tiled on host to SBUF layout: xt[p, t*128+c] = x[c, t*128+p]
    xt_d = nc.dram_tensor("xt", [C, L], F32, kind="ExternalInput")
    xyh_d = nc.dram_tensor("xyh", [C, YW], F32, kind="ExternalInput")
    mrep_d = nc.dram_tensor("mrep", [C, HALF], F32, kind="ExternalInput")
    out_d = nc.dram_tensor("out", [C, HALF], F32, kind="ExternalOutput")
    if DEBUG:
        dbg_d = {
            "d_norm2": nc.dram_tensor("d_norm2", [C, NLT], F32, kind="ExternalOutput"),
            "d_rs": nc.dram_tensor("d_rs", [C, NLT], F32, kind="ExternalOutput"),
            "d_yt": nc.dram_tensor("d_yt", [C, HALF], F32, kind="ExternalOutput"),
            "d_kn": nc.dram_tensor("d_kn", [C, L], F32, kind="ExternalOutput"),
            "d_sc0": nc.dram_tensor("d_sc0", [C, BLK], F32, kind="ExternalOutput"),
            "d_u0": nc.dram_tensor("d_u0", [C, BLK], F32, kind="ExternalOutput"),
            "d_sums0": nc.dram_tensor("d_sums0", [C, BLK], F32, kind="ExternalOutput"),
            "d_rec0": nc.dram_tensor("d_rec0", [C, BLK], F32, kind="ExternalOutput"),
        }

    with tile.TileContext(nc) as tc:
        with (
            tc.tile_pool(name="big", bufs=1) as big,
            tc.tile_pool(name="small", bufs=1) as small,
            tc.tile_pool(name="sqs", bufs=2) as sqs,
            tc.tile_pool(name="upool", bufs=8) as upool,
            tc.tile_pool(name="vpool", bufs=3) as vpool,
            tc.tile_pool(name="wpool", bufs=4) as wpool,
            tc.tile_pool(name="opool", bufs=4) as opool,
            tc.tile_pool(name="ps_sc", bufs=2, space=bass.MemorySpace.PSUM) as ps_sc,
            tc.tile_pool(name="ps_rec", bufs=1, space=bass.MemorySpace.PSUM) as ps_rec,
            tc.tile_pool(name="ps_sum", bufs=1, space=bass.MemorySpace.PSUM) as ps_sum,
        ):
            # ---- persistent SBUF tensors ----
            x_sb = big.tile([C, L], F32R, tag="x_sb")       # mm1 stationary
            xt_sb = big.tile([C, L], F32, tag="xt_sb")      # 32 tiles (128l, 128c)
            kn = big.tile([C, L], BF16, tag="kn")           # xt * rscale, l-major
            xyh_sb = big.tile([C, YW], F32, tag="xyh_sb")
            y1 = big.tile([C, YW], F32, tag="y1")
            y_t = big.tile([C, HALF], F32R, tag="y_t")
            mrep_sb = big.tile([C, HALF], F32, tag="mrep_sb")
            w_t = big.tile([C, HALF], F32, tag="w_t")       # (1-m)/9
            xm = big.tile([C, HALF], F32, tag="xm")         # x*m
            sqb = big.tile([C, 1280], F32, tag="sqb")       # xt*xt scratch
            ones_t = small.tile([C, C], BF16, tag="ones_t")
            ones_f = small.tile([C, C], F32, tag="ones_f")
            norm2 = small.tile([C, NLT], F32, tag="norm2")
            rs_a = small.tile([C, NLT], F32, tag="rs_a")
            rs_b = small.tile([C, NLT], F32, tag="rs_b")
            nt_a = small.tile([C, NLT], F32, tag="nt_a")
            eps_c = small.tile([C, 1], F32, tag="eps_c")
            shift_c = small.tile([C, 1], F32, tag="shift_c")

            # ---- input DMAs, priority-ordered across the available queues:
            # critical first pieces: xyh part A (y_t block 0), xt[0:512]
            # (norm2 chunk 0), x[0:1024] (first mm1 stationaries)
            nc.sync.dma_start(xyh_sb[:, 0:1280], xyh_d[:, 0:1280])
            nc.scalar.dma_start(xt_sb[:, 0:512], xt_d[:, 0:512])
            nc.scalar.dma_start(x_sb[:, 0:1024], x_d[:, 0:1024])
            nc.sync.dma_start(xt_sb[:, 512:1792], xt_d[:, 512:1792])
            # second wave
            nc.sync.dma_start(xyh_sb[:, 1280:YW], xyh_d[:, 1280:YW])
            nc.scalar.dma_start(xt_sb[:, 1792:3072], xt_d[:, 1792:3072])
            nc.sync.dma_start(x_sb[:, 1024:2560], x_d[:, 1024:2560])
            nc.scalar.dma_start(xt_sb[:, 3072:L], xt_d[:, 3072:L])
            nc.sync.dma_start(x_sb[:, 2560:L], x_d[:, 2560:L])
            # mask (epilogue-only use; scheduled last)
            nc.sync.dma_start(mrep_sb[:], mrep_d[:])

            # ---- constants; exp table-set load paid during the DMA window
            nc.vector.memset(ones_f[:], 1.0)
            nc.vector.tensor_copy(ones_t[:], ones_f[:])
            nc.vector.memset(eps_c[:], 1e-7)
            nc.vector.memset(shift_c[:], -SHIFT)
            warm2 = small.tile([C, 1], F32, tag="warm2")
            nc.scalar.activation(warm2[:], eps_c[:], AF.Exp)

            # ---- norm2 chunk 0 (l-tiles 0..3) on ACT during the fill
            for lt in range(4):
                scr = sqs.tile([C, C], F32, tag="sq_scratch", name="scr")
                nc.scalar.activation(
                    scr[:], xt_sb[:, lt * C:(lt + 1) * C], AF.Square,
                    accum_out=norm2[:, lt:lt + 1],
                )

            # rsqrt via bit-trick seed + 3 Newton iterations (DVE only)
            def newton(l0, l1):
                cl = slice(l0, l1)
                nc.vector.tensor_scalar(nt_a[:, cl].bitcast(I32),
                                        norm2[:, cl].bitcast(I32), 1, None,
                                        op0=ALU.logical_shift_right)
                nc.vector.tensor_scalar(rs_b[:, cl].bitcast(I32),
                                        nt_a[:, cl].bitcast(I32),
                                        -1, 0x5f3759df,
                                        op0=ALU.mult, op1=ALU.add)
                src, dst = rs_b, rs_a
                for _ in range(3):
                    nc.vector.tensor_mul(nt_a[:, cl], src[:, cl], src[:, cl])
                    nc.vector.tensor_mul(nt_a[:, cl], nt_a[:, cl], norm2[:, cl])
                    nc.vector.tensor_scalar(nt_a[:, cl], nt_a[:, cl], -0.5, 1.5,
                                            op0=ALU.mult, op1=ALU.add)
                    nc.vector.tensor_mul(dst[:, cl], src[:, cl], nt_a[:, cl])
                    src, dst = dst, src
                # odd iteration count ends with the result in rs_a

            # gpsimd squares for norm2 chunks 1..3 (l-tiles 4..13, 14..23, 24..31)
            N2CH = [(4, 14), (14, 24), (24, 32)]

            def gp_sq(ci):
                l0, l1 = N2CH[ci]
                w = (l1 - l0) * C
                nc.gpsimd.tensor_mul(sqb[:, 0:w], xt_sb[:, l0 * C:l1 * C],
                                     xt_sb[:, l0 * C:l1 * C])

            def dve_n2(ci):
                l0, l1 = N2CH[ci]
                n = l1 - l0
                v = sqb[:, 0:n * C].rearrange("p (t c) -> p t c", c=C)
                nc.vector.tensor_reduce(norm2[:, l0:l1], v, AX.X, ALU.add)
                newton(l0, l1)

            def kn_chunk(l0, l1):
                for lt in range(l0, l1):
                    nc.vector.tensor_scalar_mul(
                        kn[:, lt * C:(lt + 1) * C], xt_sb[:, lt * C:(lt + 1) * C],
                        rs_a[:, lt:lt + 1])

            # ---- y = 3x3 box filter (row filter on xyh -> y1, then col filter)
            xv = xyh_sb[:].rearrange("p (r j) -> p r j", j=64)
            yv = y1[:].rearrange("p (r j) -> p r j", j=64)

            def y1_part(r0, r1):  # rows [r0, r1) of the 34-row halo image
                a, b = r0 * 64, r1 * 64
                a1 = max(a, 1)
                nc.vector.tensor_add(y1[:, a1:b], xyh_sb[:, a1 - 1:b - 1],
                                     xyh_sb[:, a1:b])
                nc.vector.tensor_add(y1[:, a1:b - 1], y1[:, a1:b - 1],
                                     xyh_sb[:, a1 + 1:b])
                nc.vector.tensor_add(yv[:, r0:r1, 0:1], xv[:, r0:r1, 0:1],
                                     xv[:, r0:r1, 1:2])
                nc.vector.tensor_add(yv[:, r0:r1, 63:64], xv[:, r0:r1, 62:63],
                                     xv[:, r0:r1, 63:64])

            def y_t_part(c0, c1):  # y_t columns [c0, c1), needs y1 rows thru (c1/64)+2
                nc.vector.tensor_add(y_t[:, c0:c1], y1[:, c0:c1],
                                     y1[:, c0 + 64:c1 + 64])
                nc.vector.tensor_add(y_t[:, c0:c1], y_t[:, c0:c1],
                                     y1[:, c0 + 128:c1 + 128])

            # --- fill-phase emission, critical-path ordered ---
            # DVE: y for block 0 first (gates mm1), then newton chunk 0 (gates
            # exp 0), then the rest interleaves with the stream.
            y1_part(0, 19)            # y1 rows 0..18 (xyh part A)
            y_t_part(0, BLK)          # y_t block 0
            newton(0, 4)
            gp_sq(0)                  # gpsimd: squares chunk 1
            kn_chunk(0, 4)            # DVE: first kn tiles for mm2 lt 0..3

            # ---- main unified loop over (blk, lt) steps ----
            def mm1(blk, lt):
                sc = ps_sc.tile([C, BLK], F32, tag="sc", name="sc")
                for h2 in range(2):
                    cs = blk * BLK + h2 * 512
                    nc.tensor.matmul(
                        sc[:, h2 * 512:(h2 + 1) * 512],
                        x_sb[:, lt * C:(lt + 1) * C],
                        y_t[:, cs:cs + 512],
                        start=True, stop=True,
                    )
                return sc

            state = {}

            def emit_ones(blk, w, j, n_groups):
                sums = state[("sums", blk)]
                for h2 in range(2):
                    nc.tensor.matmul(
                        sums[:, h2 * 512:(h2 + 1) * 512],
                        ones_t[:],
                        w[:, h2 * 512:(h2 + 1) * 512],
                        start=(j == 0), stop=(j == n_groups - 1),
                    )

            def epilogue(blk, part):
                # part 0: R/Rm/ob for both 512-chunks (frees rec asap)
                # part 1: + xm and the output DMAs
                rec = state[("rec", blk)]
                sums = state[("sums", blk)]
                if part == 0:
                    if DEBUG and blk == 0:
                        dsm = big.tile([C, BLK], F32, tag="dsm", name="dsm")
                        nc.vector.tensor_copy(dsm[:], sums[:])
                        nc.sync.dma_start(dbg_d["d_sums0"][:], dsm[:])
                        drc = big.tile([C, BLK], F32, tag="drc", name="drc")
                        nc.vector.tensor_copy(drc[:], rec[:])
                        nc.sync.dma_start(dbg_d["d_rec0"][:], drc[:])
                    obs = []
                    for h2 in range(2):
                        cs = blk * BLK + h2 * 512
                        sl = slice(h2 * 512, (h2 + 1) * 512)
                        R = opool.tile([C, 512], F32, tag="R", name="R")
                        nc.vector.reciprocal_approx_fast(R[:], sums[:, sl])
                        Rm = opool.tile([C, 512], F32, tag="Rm", name="Rm")
                        nc.vector.tensor_mul(Rm[:], R[:], w_t[:, cs:cs + 512])
                        ob = opool.tile([C, 512], F32, tag=f"ob{h2}", name="ob")
                        nc.vector.tensor_mul(ob[:], rec[:, sl], Rm[:])
                        obs.append(ob)
                    state[("obs", blk)] = obs
                else:
                    obs = state.pop(("obs", blk))
                    for h2 in range(2):
                        cs = blk * BLK + h2 * 512
                        ob = obs[h2]
                        nc.vector.tensor_add(ob[:], ob[:], xm[:, cs:cs + 512])
                        nc.sync.dma_start(out_d[:, cs:cs + 512], ob[:])

            MM2_LAG = 5  # steps mm2 trails its exp in block 1's head
            pend_mm2 = []

            def mm2(blk, lt, u):
                rec = state[("rec", blk)]
                for h2 in range(2):
                    nc.tensor.matmul(
                        rec[:, h2 * 512:(h2 + 1) * 512],
                        kn[:, lt * C:(lt + 1) * C],
                        u[:, h2 * 512:(h2 + 1) * 512],
                        start=(lt == 0), stop=(lt == NLT - 1),
                    )

            # number of ones-mm accumulation groups per block:
            # 7 tree groups of 4 + last 4 l-tiles direct
            N_GROUPS = NLT // 4 - 1 + 4

            state[("sc", 0)] = mm1(0, 0)
            for step in range(NSTEP):
                blk, lt = divmod(step, NLT)
                if lt == 0:
                    state[("rec", blk)] = ps_rec.tile([C, BLK], F32, tag="rec", name="rec")
                    state[("sums", blk)] = ps_sum.tile([C, BLK], F32, tag="sums", name="sums")
                    state[("uq", blk)] = None      # u_prev for the tree
                    state[("vq", blk)] = None      # v_prev for the tree
                    state[("wq", blk)] = []        # lagged ones-mm queue
                    state[("wi", blk)] = 0

                # mm1 one step ahead (keeps ACT fed; emitted before mm2/ones)
                if step + 1 < NSTEP:
                    state[("sc", step + 1)] = mm1(*divmod(step + 1, NLT))

                # exp: u = Exp(sc * rscale[lt] - 20), bf16 out
                state_dbg_sc = state[("sc", step)]
                u = upool.tile([C, BLK], BF16, tag="u", name="u")
                nc.scalar.activation(u[:], state.pop(("sc", step))[:], AF.Exp,
                                     bias=shift_c[:],
                                     scale=rs_a[:, lt:lt + 1])

                if DEBUG and step == 0:
                    dsc = big.tile([C, BLK], F32, tag="dsc", name="dsc")
                    nc.vector.tensor_copy(dsc[:], state_dbg_sc[:])
                    nc.sync.dma_start(dbg_d["d_sc0"][:], dsc[:])
                    du = big.tile([C, BLK], F32, tag="du", name="du")
                    nc.vector.tensor_copy(du[:], u[:])
                    nc.sync.dma_start(dbg_d["d_u0"][:], du[:])

                # mm2 lagged: block 0's head waits for the kn chunk emitted
                # in the same step's extras; block 1's head waits for the
                # block-0 epilogue to free the rec PSUM buffer
                pend_mm2.append((blk, lt, u))
                if blk == 0:
                    lag = 2 if lt < 6 else 0
                else:
                    lag = max(0, min(MM2_LAG, 12 - lt))
                while len(pend_mm2) > lag:
                    mm2(*pend_mm2.pop(0))

                # column sums: 2-level pairwise bf16 tree on DVE; PE finishes
                # with a lagged ones-mm per group of 4. Last 4 l-tiles: direct
                # PE ones-mm (shortens the tail before the epilogue).
                if lt >= NLT - 4:
                    j = state[("wi", blk)]
                    emit_ones(blk, u, j, N_GROUPS)
                    state[("wi", blk)] = j + 1
                elif lt % 2 == 0:
                    state[("uq", blk)] = u
                else:
                    v = vpool.tile([C, BLK], BF16, tag="v", name="v")
                    nc.vector.tensor_add(v[:], state[("uq", blk)][:], u[:])
                    if state[("vq", blk)] is None:
                        state[("vq", blk)] = v
                    else:
                        w = wpool.tile([C, BLK], BF16, tag="w", name="w")
                        nc.vector.tensor_add(w[:], state[("vq", blk)][:], v[:])
                        state[("vq", blk)] = None
                        state[("wq", blk)].append((w, state[("wi", blk)]))
                        state[("wi", blk)] = state[("wi", blk)] + 1
                        if len(state[("wq", blk)]) > 2:
                            emit_ones(blk, *state[("wq", blk)].pop(0),
                                      N_GROUPS)
                if lt == NLT - 5:
                    for w, j in state[("wq", blk)]:
                        emit_ones(blk, w, j, N_GROUPS)
                    state[("wq", blk)] = []

                # interleaved prologue/epilogue extras (block 0 stream feeds
                # later chunks; block 1 stream drains block 0's epilogue)
                if blk == 0:
                    if lt == 1:
                        y1_part(19, 34)       # y1 rows 19..33 (xyh part B)
                    elif lt == 2:
                        y_t_part(BLK, HALF)   # y_t block 1
                    elif lt == 3:
                        dve_n2(0)             # norm2+newton l-tiles 4..13
                    elif lt == 4:
                        gp_sq(1)
                        kn_chunk(4, 8)
                    elif lt == 6:
                        kn_chunk(8, 14)
                    elif lt == 9:
                        dve_n2(1)             # l-tiles 14..23
                    elif lt == 10:
                        gp_sq(2)
                    elif lt == 11:
                        kn_chunk(14, 20)
                    elif lt == 14:
                        dve_n2(2)             # l-tiles 24..31
                    elif lt == 15:
                        kn_chunk(20, 24)
                    elif lt == 17:
                        kn_chunk(24, NLT)
                    elif lt == 19:
                        nc.vector.tensor_scalar(w_t[:], mrep_sb[:],
                                                -1.0 / 9.0, 1.0 / 9.0,
                                                op0=ALU.mult, op1=ALU.add)
                    elif lt == 21:
                        # xm = x * m on gpsimd (xyh center = this half of x)
                        nc.gpsimd.tensor_mul(xm[:], xyh_sb[:, 64:64 + HALF],
                                             mrep_sb[:])
                else:
                    if lt == 1:
                        epilogue(0, 0)
                    elif lt == 3:
                        epilogue(0, 1)

            # drain: block 1 epilogue
            epilogue(1, 0)
            epilogue(1, 1)
            if DEBUG:
                nc.sync.dma_start(dbg_d["d_norm2"][:], norm2[:])
                nc.sync.dma_start(dbg_d["d_rs"][:], rs_a[:])
                dyt = big.tile([C, HALF], F32, tag="dyt", name="dyt")
                nc.vector.tensor_copy(dyt[:], y_t[:])
                nc.sync.dma_start(dbg_d["d_yt"][:], dyt[:])
                dkn = big.tile([C, L], F32, tag="dkn", name="dkn")
                nc.vector.tensor_copy(dkn[:], kn[:])
                nc.sync.dma_start(dbg_d["d_kn"][:], dkn[:])

    nc.finalize()
    return nc


def _get_program():
    if "nc" not in _CACHE:
        _CACHE["nc"] = _build_program()
    return _CACHE["nc"]


def _make_in_maps(fg, mk):
    in_maps = []
    for core in range(8):
        b, h = core // 2, core % 2
        x = np.ascontiguousarray(fg[b].reshape(C, L))
        xt = np.ascontiguousarray(
            x.reshape(C, L // C, C).transpose(2, 1, 0).reshape(C, L))
        xi = fg[b].reshape(C, 64, 64)
        rows = np.zeros((C, 34, 64), np.float32)
        r0 = 32 * h - 1
        lo, hi = max(0, r0), min(64, r0 + 34)
        rows[:, lo - r0:hi - r0, :] = xi[:, lo:hi, :]
        xyh = np.ascontiguousarray(rows.reshape(C, YW))
        mrow = mk[b].reshape(1, L)[:, h * HALF:(h + 1) * HALF]
        mrep = np.ascontiguousarray(np.broadcast_to(mrow, (C, HALF)))
        in_maps.append({"x": x, "xt": xt, "xyh": xyh, "mrep": mrep})
    return in_maps


def kernel(foreground, mask):
    fg = np.ascontiguousarray(np.asarray(foreground, dtype=np.float32))
    mk = np.ascontiguousarray(np.asarray(mask, dtype=np.float32))
    nc = _get_program()
    in_maps = _make_in_maps(fg, mk)

    from concourse.bass_utils import run_bass_kernel_spmd
    res = run_bass_kernel_spmd(nc, in_maps, core_ids=list(range(8)))

    out = np.empty((4, C, L), np.float32)
    for core in range(8):
        b, h = core // 2, core % 2
        out[b][:, h * HALF:(h + 1) * HALF] = res.results[core]["out"]
    if DEBUG:
        _CACHE["debug"] = [dict(r) for r in res.results]
    return out.reshape(4, C, 64, 64)


# revision 17
# speedup vs baseline: 1.0310x; 1.0310x over previous
"""Trainium2 Bass kernel for CAttention (contextual attention).

Math (per batch element, derived from the reference):
    x:    (c=128, h=64, w=64), flat (128, 4096); m: (1, 4096)
    k    = normalize_rows(x.reshape(c, hw).T + eps)          # (4096, 128)
    y    = 3x3 zero-padded box filter of x                   # (128, 4096)
    S    = k @ y                                             # (4096 l, 4096 ij)
    att  = softmax over l (per column); constant-shift trick: softmax needs no
           per-column max because S is bounded (|S| <= ~34, col max >= ~11):
           u = exp(S - 20), att = u / colsum(u)
    rec  = k.T @ att                                         # (128, 4096)
    out  = rec * (1-m)/9 + x*m
    (eps=1e-7 is dropped on-chip: its effect is ~1e-7 relative, far below the
     accuracy gate)

Sharding: pure data parallel over batch (4) x output-column halves (2) = 8
cores, zero cross-core communication. Each core: full l = 4096, its 2048
output columns.

v2 design (from the v1 trace: ACT exp stream = the bottleneck spine; fill was
29us, drain 17us):
  - The 64 exps (ACT, ~1.18us each, dtype-independent rate) are the hard
    floor (~76us). Everything else is arranged to fit underneath and the
    fill/drain around the stream is minimized.
  - exp outputs bf16: DVE sum-tree gets 2x throughput, SBUF traffic halves.
  - norm2 = sum_c x^2: first 4 l-tiles via ACT Square(+accum) during fill;
    rest via gpsimd x*x multiplies + DVE segmented tensor_reduce (keeps the
    ACT free for exps).
  - mm1 stationary is x itself (dram/sbuf tiles typed f32r, no eps pass).
  - software-pipelined unified 64-step loop: mm1 emitted one step ahead of
    its exp; mm2 of the second block lagged a few steps so the block-0
    epilogue (which holds the single rec PSUM buffer) never stalls block-1
    mm1s (ACT never starves).
  - mask row shipped as [1, 2048] and broadcast to 128 partitions by DMA.
"""

import numpy as np

SHIFT = 20.0
C = 128          # channels
L = 4096         # spatial locations (l axis)
HALF = 2048      # output columns per core
BLK = 1024       # ij block (psum-bank sized: 2 banks)
NLT = 32         # l tiles of 128
YW = 2176        # xyh width: 34 padded image rows x 64
NSTEP = 2 * NLT  # unified (blk, lt) steps

_CACHE = {}
DEBUG = False


def _build_program():
    import concourse.bass as bass
    import concourse.bacc as bacc
    import concourse.tile as tile
    import concourse.mybir as mybir

    F32 = mybir.dt.float32
    F32R = mybir.dt.float32r
    BF16 = mybir.dt.bfloat16
    AF = mybir.ActivationFunctionType
    ALU = mybir.AluOpType
    AX = mybir.AxisListType
    I32 = mybir.dt.int32

    nc = bacc.Bacc("TRN2", target_bir_lowering=False, num_swdge_queues=4)

    # x typed f32r end-to-end: used only as the mm1 stationary operand
    x_d = nc.dram_tensor("x", [C, L], F32R, kind="ExternalInput")
    # xt pre-tiled on host to SBUF layout: xt[p, t*128+c] = x[c, t*128+p]
    xt_d = nc.dram_tensor("xt", [C, L], F32, kind="ExternalInput")
    xyh_d = nc.dram_tensor("xyh", [C, YW], F32, kind="ExternalInput")
    mrow_d = nc.dram_tensor("mrow", [1, HALF], F32R, kind="ExternalInput")
    out_d = nc.dram_tensor("out", [C, HALF], F32, kind="ExternalOutput")
    if DEBUG:
        dbg_d = {
            "d_norm2": nc.dram_tensor("d_norm2", [C, NLT], F32, kind="ExternalOutput"),
            "d_rs": nc.dram_tensor("d_rs", [C, NLT], F32, kind="ExternalOutput"),
            "d_yt": nc.dram_tensor("d_yt", [C, HALF], F32, kind="ExternalOutput"),
            "d_kn": nc.dram_tensor("d_kn", [C, L], F32, kind="ExternalOutput"),
            "d_sc0": nc.dram_tensor("d_sc0", [C, BLK], F32, kind="ExternalOutput"),
            "d_u0": nc.dram_tensor("d_u0", [C, BLK], F32, kind="ExternalOutput"),
            "d_sums0": nc.dram_tensor("d_sums0", [C, BLK], F32, kind="ExternalOutput"),
            "d_rec0": nc.dram_tensor("d_rec0", [C, BLK], F32, kind="ExternalOutput"),
        }

    with tile.TileContext(nc) as tc:
        with (
            tc.tile_pool(name="big", bufs=1) as big,
            tc.tile_pool(name="small", bufs=1) as small,
            tc.tile_pool(name="sqs", bufs=2) as sqs,
            tc.tile_pool(name="upool", bufs=8) as upool,
            tc.tile_pool(name="vpool", bufs=3) as vpool,
            tc.tile_pool(name="wpool", bufs=4) as wpool,
            tc.tile_pool(name="opool", bufs=4) as opool,
            tc.tile_pool(name="ps_sc", bufs=2, space=bass.MemorySpace.PSUM) as ps_sc,
            tc.tile_pool(name="ps_rec", bufs=1, space=bass.MemorySpace.PSUM) as ps_rec,
            tc.tile_pool(name="ps_sum", bufs=1, space=bass.MemorySpace.PSUM) as ps_sum,
        ):
            # ---- persistent SBUF tensors ----
            x_sb = big.tile([C, L], F32R, tag="x_sb")       # mm1 stationary
            xt_sb = big.tile([C, L], F32, tag="xt_sb")      # 32 tiles (128l, 128c)
            kn = big.tile([C, L], BF16, tag="kn")           # xt * rscale, l-major
            xyh_sb = big.tile([C, YW], F32, tag="xyh_sb")
            y1 = big.tile([C, YW], F32, tag="y1")
            y_t = big.tile([C, HALF], F32R, tag="y_t")
            mrow_sb = big.tile([1, HALF], F32R, tag="mrow_sb")
            ones_rf = big.tile([1, C], F32, tag="ones_rf")
            ones_row = big.tile([1, C], F32R, tag="ones_row")
            w_t = big.tile([C, HALF], F32, tag="w_t")       # (1-m)/9
            xm = big.tile([C, HALF], F32, tag="xm")         # x*m
            sqb = big.tile([C, 1280], F32, tag="sqb")       # xt*xt scratch
            ones_t = small.tile([C, C], BF16, tag="ones_t")
            ones_f = small.tile([C, C], F32, tag="ones_f")
            norm2 = small.tile([C, NLT], F32, tag="norm2")
            rs_a = small.tile([C, NLT], F32, tag="rs_a")
            rs_b = small.tile([C, NLT], F32, tag="rs_b")
            nt_a = small.tile([C, NLT], F32, tag="nt_a")
            eps_c = small.tile([C, 1], F32, tag="eps_c")
            shift_c = small.tile([C, 1], F32, tag="shift_c")

            # ---- input DMAs. The two HWDGE queues (sync, scalar) fair-share
            # ~100-150 GB/s each and are FIFO per queue, so wave 1 carries ONLY
            # what gates the first exps; everything else queues behind it.
            nc.sync.dma_start(xyh_sb[:, 0:1216], xyh_d[:, 0:1216])
            nc.scalar.dma_start(xt_sb[:, 0:768], xt_d[:, 0:768])
            nc.scalar.dma_start(x_sb[:, 0:512], x_d[:, 0:512])
            nc.scalar.dma_start(mrow_sb[:], mrow_d[:])
            # wave 2
            nc.sync.dma_start(xyh_sb[:, 1216:YW], xyh_d[:, 1216:YW])
            nc.scalar.dma_start(xt_sb[:, 768:2048], xt_d[:, 768:2048])
            nc.sync.dma_start(x_sb[:, 512:2048], x_d[:, 512:2048])
            # wave 3
            nc.scalar.dma_start(xt_sb[:, 2048:3328], xt_d[:, 2048:3328])
            nc.sync.dma_start(x_sb[:, 2048:L], x_d[:, 2048:L])
            nc.scalar.dma_start(xt_sb[:, 3328:L], xt_d[:, 3328:L])

            # ---- constants; exp table-set load paid during the DMA window
            nc.vector.memset(ones_f[:], 1.0)
            nc.vector.tensor_copy(ones_t[:], ones_f[:])
            nc.vector.memset(eps_c[:], 1e-7)
            nc.vector.memset(shift_c[:], -SHIFT)
            nc.vector.memset(ones_rf[:], 1.0)
            nc.vector.tensor_copy(ones_row[:], ones_rf[:])
            warm2 = small.tile([C, 1], F32, tag="warm2")
            nc.scalar.activation(warm2[:], eps_c[:], AF.Exp)

            # ---- norm2 chunk 0 (l-tiles 0..5) on ACT during the fill
            for lt in range(6):
                scr = sqs.tile([C, C], F32, tag="sq_scratch", name="scr")
                nc.scalar.activation(
                    scr[:], xt_sb[:, lt * C:(lt + 1) * C], AF.Square,
                    accum_out=norm2[:, lt:lt + 1],
                )

            # rsqrt via bit-trick seed + 3 Newton iterations (DVE only)
            def newton(l0, l1):
                cl = slice(l0, l1)
                nc.vector.tensor_scalar(nt_a[:, cl].bitcast(I32),
                                        norm2[:, cl].bitcast(I32), 1, None,
                                        op0=ALU.logical_shift_right)
                nc.vector.tensor_scalar(rs_b[:, cl].bitcast(I32),
                                        nt_a[:, cl].bitcast(I32),
                                        -1, 0x5f3759df,
                                        op0=ALU.mult, op1=ALU.add)
                src, dst = rs_b, rs_a
                for _ in range(3):
                    nc.vector.tensor_mul(nt_a[:, cl], src[:, cl], src[:, cl])
                    nc.vector.tensor_mul(nt_a[:, cl], nt_a[:, cl], norm2[:, cl])
                    nc.vector.tensor_scalar(nt_a[:, cl], nt_a[:, cl], -0.5, 1.5,
                                            op0=ALU.mult, op1=ALU.add)
                    nc.vector.tensor_mul(dst[:, cl], src[:, cl], nt_a[:, cl])
                    src, dst = dst, src
                # odd iteration count ends with the result in rs_a

            # norm2 chunks: A = l-tiles 6..16 squared on DVE (early deadline),
            # B = 16..26 and C = 26..32 squared on gpsimd (late deadlines)
            N2CH = [(6, 16), (16, 26), (26, 32)]

            def gp_sq(ci):
                l0, l1 = N2CH[ci]
                w = (l1 - l0) * C
                nc.gpsimd.tensor_mul(sqb[:, 0:w], xt_sb[:, l0 * C:l1 * C],
                                     xt_sb[:, l0 * C:l1 * C])

            def dve_sq(ci):
                l0, l1 = N2CH[ci]
                w = (l1 - l0) * C
                nc.vector.tensor_mul(sqb[:, 0:w], xt_sb[:, l0 * C:l1 * C],
                                     xt_sb[:, l0 * C:l1 * C])

            def dve_n2(ci):
                l0, l1 = N2CH[ci]
                n = l1 - l0
                v = sqb[:, 0:n * C].rearrange("p (t c) -> p t c", c=C)
                nc.vector.tensor_reduce(norm2[:, l0:l1], v, AX.X, ALU.add)
                newton(l0, l1)

            def kn_chunk(l0, l1):
                for lt in range(l0, l1):
                    nc.vector.tensor_scalar_mul(
                        kn[:, lt * C:(lt + 1) * C], xt_sb[:, lt * C:(lt + 1) * C],
                        rs_a[:, lt:lt + 1])

            # ---- y = 3x3 box filter (row filter on xyh -> y1, then col filter)
            xv = xyh_sb[:].rearrange("p (r j) -> p r j", j=64)
            yv = y1[:].rearrange("p (r j) -> p r j", j=64)

            def y1_part(r0, r1):  # rows [r0, r1) of the 34-row halo image
                a, b = r0 * 64, r1 * 64
                a1 = max(a, 1)
                nc.vector.tensor_add(y1[:, a1:b], xyh_sb[:, a1 - 1:b - 1],
                                     xyh_sb[:, a1:b])
                nc.vector.tensor_add(y1[:, a1:b - 1], y1[:, a1:b - 1],
                                     xyh_sb[:, a1 + 1:b])
                nc.vector.tensor_add(yv[:, r0:r1, 0:1], xv[:, r0:r1, 0:1],
                                     xv[:, r0:r1, 1:2])
                nc.vector.tensor_add(yv[:, r0:r1, 63:64], xv[:, r0:r1, 62:63],
                                     xv[:, r0:r1, 63:64])

            def y_t_part(c0, c1):  # y_t columns [c0, c1), needs y1 rows thru (c1/64)+2
                nc.vector.tensor_add(y_t[:, c0:c1], y1[:, c0:c1],
                                     y1[:, c0 + 64:c1 + 64])
                nc.vector.tensor_add(y_t[:, c0:c1], y_t[:, c0:c1],
                                     y1[:, c0 + 128:c1 + 128])

            # --- fill-phase emission, critical-path ordered ---
            # DVE: y for block 0 first (gates mm1), then newton chunk 0 (gates
            # exp 0), then the rest interleaves with the stream.
            y1_part(0, 19)            # y1 rows 0..18 (xyh part A)
            y_t_part(0, BLK)          # y_t block 0
            newton(0, 6)
            kn_chunk(0, 6)            # DVE: kn tiles for mm2 lt 0..5
            gp_sq(1)                  # gpsimd: squares l-tiles 16..26

            # mask broadcast: mb = ones_row.T @ mrow (K=1 matmul) -> PSUM,
            # immediately converted to w_t = (1-m)/9 and xm = x*m so the
            # ps_sum buffer frees long before the first real sums tile
            def mask_half(half):
                t = ps_sum.tile([C, BLK], F32, tag="sums", name="mb")
                cs = half * BLK
                for h2 in range(2):
                    c2 = cs + h2 * 512
                    nc.tensor.matmul(
                        t[:, h2 * 512:(h2 + 1) * 512],
                        ones_row[:],
                        mrow_sb[:, c2:c2 + 512],
                        start=True, stop=True,
                    )
                nc.vector.tensor_scalar(w_t[:, cs:cs + BLK], t[:],
                                        -1.0 / 9.0, 1.0 / 9.0,
                                        op0=ALU.mult, op1=ALU.add)
                nc.vector.tensor_mul(xm[:, cs:cs + BLK],
                                     xyh_sb[:, 64 + cs:64 + cs + BLK], t[:])

            mask_half(0)

            # ---- main unified loop over (blk, lt) steps ----
            def mm1(blk, lt):
                sc = ps_sc.tile([C, BLK], F32, tag="sc", name="sc")
                for h2 in range(2):
                    cs = blk * BLK + h2 * 512
                    nc.tensor.matmul(
                        sc[:, h2 * 512:(h2 + 1) * 512],
                        x_sb[:, lt * C:(lt + 1) * C],
                        y_t[:, cs:cs + 512],
                        start=True, stop=True,
                    )
                return sc

            state = {}

            def emit_ones(blk, w, j, n_groups):
                sums = state[("sums", blk)]
                for h2 in range(2):
                    nc.tensor.matmul(
                        sums[:, h2 * 512:(h2 + 1) * 512],
                        ones_t[:],
                        w[:, h2 * 512:(h2 + 1) * 512],
                        start=(j == 0), stop=(j == n_groups - 1),
                    )

            def epilogue(blk, part):
                # part 0: R/Rm/ob for both 512-chunks (frees rec asap)
                # part 1: + xm and the output DMAs
                rec = state[("rec", blk)]
                sums = state[("sums", blk)]
                if part == 0:
                    if DEBUG and blk == 0:
                        dsm = big.tile([C, BLK], F32, tag="dsm", name="dsm")
                        nc.vector.tensor_copy(dsm[:], sums[:])
                        nc.sync.dma_start(dbg_d["d_sums0"][:], dsm[:])
                        drc = big.tile([C, BLK], F32, tag="drc", name="drc")
                        nc.vector.tensor_copy(drc[:], rec[:])
                        nc.sync.dma_start(dbg_d["d_rec0"][:], drc[:])
                    obs = []
                    for h2 in range(2):
                        cs = blk * BLK + h2 * 512
                        sl = slice(h2 * 512, (h2 + 1) * 512)
                        R = opool.tile([C, 512], F32, tag="R", name="R")
                        nc.vector.reciprocal_approx_fast(R[:], sums[:, sl])
                        Rm = opool.tile([C, 512], F32, tag="Rm", name="Rm")
                        nc.vector.tensor_mul(Rm[:], R[:], w_t[:, cs:cs + 512])
                        ob = opool.tile([C, 512], F32, tag=f"ob{h2}", name="ob")
                        nc.vector.tensor_mul(ob[:], rec[:, sl], Rm[:])
                        obs.append(ob)
                    state[("obs", blk)] = obs
                else:
                    obs = state.pop(("obs", blk))
                    for h2 in range(2):
                        cs = blk * BLK + h2 * 512
                        ob = obs[h2]
                        nc.vector.tensor_add(ob[:], ob[:], xm[:, cs:cs + 512])
                        nc.sync.dma_start(out_d[:, cs:cs + 512], ob[:])

            MM2_LAG = 5  # steps mm2 trails its exp in block 1's head
            pend_mm2 = []

            def mm2(blk, lt, u):
                rec = state[("rec", blk)]
                for h2 in range(2):
                    nc.tensor.matmul(
                        rec[:, h2 * 512:(h2 + 1) * 512],
                        kn[:, lt * C:(lt + 1) * C],
                        u[:, h2 * 512:(h2 + 1) * 512],
                        start=(lt == 0), stop=(lt == NLT - 1),
                    )

            # number of ones-mm accumulation groups per block:
            # 7 tree groups of 4 + last 4 l-tiles direct
            N_GROUPS = NLT // 4 - 1 + 4

            state[("sc", 0)] = mm1(0, 0)
            for step in range(NSTEP):
                blk, lt = divmod(step, NLT)
                if lt == 0:
                    state[("rec", blk)] = ps_rec.tile([C, BLK], F32, tag="rec", name="rec")
                    state[("sums", blk)] = ps_sum.tile([C, BLK], F32, tag="sums", name="sums")
                    state[("uq", blk)] = None      # u_prev for the tree
                    state[("vq", blk)] = None      # v_prev for the tree
                    state[("wq", blk)] = []        # lagged ones-mm queue
                    state[("wi", blk)] = 0

                # mm1 one step ahead (keeps ACT fed; emitted before mm2/ones)
                if step + 1 < NSTEP:
                    state[("sc", step + 1)] = mm1(*divmod(step + 1, NLT))

                # exp: u = Exp(sc * rscale[lt] - 20), bf16 out
                state_dbg_sc = state[("sc", step)]
                u = upool.tile([C, BLK], BF16, tag="u", name="u")
                nc.scalar.activation(u[:], state.pop(("sc", step))[:], AF.Exp,
                                     bias=shift_c[:],
                                     scale=rs_a[:, lt:lt + 1])

                if DEBUG and step == 0:
                    dsc = big.tile([C, BLK], F32, tag="dsc", name="dsc")
                    nc.vector.tensor_copy(dsc[:], state_dbg_sc[:])
                    nc.sync.dma_start(dbg_d["d_sc0"][:], dsc[:])
                    du = big.tile([C, BLK], F32, tag="du", name="du")
                    nc.vector.tensor_copy(du[:], u[:])
                    nc.sync.dma_start(dbg_d["d_u0"][:], du[:])

                # mm2 lagged: block 0's head waits for the kn chunk emitted
                # in the same step's extras; block 1's head waits for the
                # block-0 epilogue to free the rec PSUM buffer
                pend_mm2.append((blk, lt, u))
                if blk == 0:
                    lag = 2 if lt < 8 else 0
                else:
                    lag = max(0, min(MM2_LAG, 12 - lt))
                while len(pend_mm2) > lag:
                    mm2(*pend_mm2.pop(0))

                # column sums: 2-level pairwise bf16 tree on DVE; PE finishes
                # with a lagged ones-mm per group of 4. Last 4 l-tiles: direct
                # PE ones-mm (shortens the tail before the epilogue).
                if lt >= NLT - 4:
                    j = state[("wi", blk)]
                    emit_ones(blk, u, j, N_GROUPS)
                    state[("wi", blk)] = j + 1
                elif lt % 2 == 0:
                    state[("uq", blk)] = u
                else:
                    v = vpool.tile([C, BLK], BF16, tag="v", name="v")
                    nc.vector.tensor_add(v[:], state[("uq", blk)][:], u[:])
                    if state[("vq", blk)] is None:
                        state[("vq", blk)] = v
                    else:
                        w = wpool.tile([C, BLK], BF16, tag="w", name="w")
                        nc.vector.tensor_add(w[:], state[("vq", blk)][:], v[:])
                        state[("vq", blk)] = None
                        state[("wq", blk)].append((w, state[("wi", blk)]))
                        state[("wi", blk)] = state[("wi", blk)] + 1
                        if len(state[("wq", blk)]) > 2:
                            emit_ones(blk, *state[("wq", blk)].pop(0),
                                      N_GROUPS)
                if lt == NLT - 5:
                    for w, j in state[("wq", blk)]:
                        emit_ones(blk, w, j, N_GROUPS)
                    state[("wq", blk)] = []

                # interleaved prologue/epilogue extras (block 0 stream feeds
                # later chunks; block 1 stream drains block 0's epilogue)
                if blk == 0:
                    if lt == 1:
                        dve_sq(0)             # squares l-tiles 6..15 on DVE
                        dve_n2(0)
                    elif lt == 3:
                        kn_chunk(6, 12)
                        y1_part(19, 34)       # y1 rows 19..33 (xyh part B)
                    elif lt == 4:
                        y_t_part(BLK, HALF)   # y_t block 1
                    elif lt == 5:
                        kn_chunk(12, 16)
                        gp_sq(2)              # gpsimd: squares l-tiles 26..31
                    elif lt == 7:
                        dve_n2(1)             # reduce+newton l-tiles 16..25
                    elif lt == 8:
                        kn_chunk(16, 22)
                    elif lt == 11:
                        dve_n2(2)             # reduce+newton l-tiles 26..31
                    elif lt == 12:
                        kn_chunk(22, 26)
                    elif lt == 14:
                        kn_chunk(26, NLT)
                    elif lt == 2:
                        mask_half(1)
                else:
                    if lt == 1:
                        epilogue(0, 0)
                    elif lt == 3:
                        epilogue(0, 1)

            # drain: block 1 epilogue
            epilogue(1, 0)
            epilogue(1, 1)
            if DEBUG:
                nc.sync.dma_start(dbg_d["d_norm2"][:], norm2[:])
                nc.sync.dma_start(dbg_d["d_rs"][:], rs_a[:])
                dyt = big.tile([C, HALF], F32, tag="dyt", name="dyt")
                nc.vector.tensor_copy(dyt[:], y_t[:])
                nc.sync.dma_start(dbg_d["d_yt"][:], dyt[:])
                dkn = big.tile([C, L], F32, tag="dkn", name="dkn")
                nc.vector.tensor_copy(dkn[:], kn[:])
                nc.sync.dma_start(dbg_d["d_kn"][:], dkn[:])

    nc.finalize()
    return nc


def _get_program():
    if "nc" not in _CACHE:
        _CACHE["nc"] = _build_program()
    return _CACHE["nc"]


def _make_in_maps(fg, mk):
    in_maps = []
    for core in range(8):
        b, h = core // 2, core % 2
        x = np.ascontiguousarray(fg[b].reshape(C, L))
        xt = np.ascontiguousarray(
            x.reshape(C, L // C, C).transpose(2, 1, 0).reshape(C, L))
        xi = fg[b].reshape(C, 64, 64)
        rows = np.zeros((C, 34, 64), np.float32)
        r0 = 32 * h - 1
        lo, hi = max(0, r0), min(64, r0 + 34)
        rows[:, lo - r0:hi - r0, :] = xi[:, lo:hi, :]
        xyh = np.ascontiguousarray(rows.reshape(C, YW))
        mrow = np.ascontiguousarray(
            mk[b].reshape(1, L)[:, h * HALF:(h + 1) * HALF])
        in_maps.append({"x": x, "xt": xt, "xyh": xyh, "mrow": mrow})
    return in_maps


def kernel(foreground, mask):
    fg = np.ascontiguousarray(np.asarray(foreground, dtype=np.float32))
    mk = np.ascontiguousarray(np.asarray(mask, dtype=np.float32))
    nc = _get_program()
    in_maps = _make_in_maps(fg, mk)

    from concourse.bass_utils import run_bass_kernel_spmd
    res = run_bass_kernel_spmd(nc, in_maps, core_ids=list(range(8)))

    out = np.empty((4, C, L), np.float32)
    for core in range(8):
        b, h = core // 2, core % 2
        out[b][:, h * HALF:(h + 1) * HALF] = res.results[core]["out"]
    if DEBUG:
        _CACHE["debug"] = [dict(r) for r in res.results]
    return out.reshape(4, C, 64, 64)


# revision 18
# speedup vs baseline: 1.2614x; 1.2235x over previous
"""Trainium2 Bass kernel for CAttention (contextual attention).

Math (per batch element, derived from the reference):
    x:    (c=128, h=64, w=64), flat (128, 4096); m: (1, 4096)
    k    = normalize_rows(x.reshape(c, hw).T + eps)          # (4096, 128)
    y    = 3x3 zero-padded box filter of x                   # (128, 4096)
    S    = k @ y                                             # (4096 l, 4096 ij)
    att  = softmax over l (per column); constant-shift trick: softmax needs no
           per-column max because S is bounded (|S| <= ~34, col max >= ~11):
           u = exp(S - 20), att = u / colsum(u)
    rec  = k.T @ att                                         # (128, 4096)
    out  = rec * (1-m)/9 + x*m
    (eps=1e-7 is dropped on-chip: its effect is ~1e-7 relative, far below the
     accuracy gate)

Sharding: pure data parallel over batch (4) x output-column halves (2) = 8
cores, zero cross-core communication. Each core: full l = 4096, its 2048
output columns.

v2 design (from the v1 trace: ACT exp stream = the bottleneck spine; fill was
29us, drain 17us):
  - The 64 exps (ACT, ~1.18us each, dtype-independent rate) are the hard
    floor (~76us). Everything else is arranged to fit underneath and the
    fill/drain around the stream is minimized.
  - exp outputs bf16: DVE sum-tree gets 2x throughput, SBUF traffic halves.
  - norm2 = sum_c x^2: first 4 l-tiles via ACT Square(+accum) during fill;
    rest via gpsimd x*x multiplies + DVE segmented tensor_reduce (keeps the
    ACT free for exps).
  - mm1 stationary is x itself (dram/sbuf tiles typed f32r, no eps pass).
  - software-pipelined unified 64-step loop: mm1 emitted one step ahead of
    its exp; mm2 of the second block lagged a few steps so the block-0
    epilogue (which holds the single rec PSUM buffer) never stalls block-1
    mm1s (ACT never starves).
  - mask row shipped as [1, 2048] and broadcast to 128 partitions by DMA.
"""

import numpy as np

SHIFT = 20.0
C = 128          # channels
L = 4096         # spatial locations (l axis)
HALF = 2048      # output columns per core
BLK = 1024       # ij block (psum-bank sized: 2 banks)
NLT = 32         # l tiles of 128
YW = 2176        # xyh width: 34 padded image rows x 64
NSTEP = 2 * NLT  # unified (blk, lt) steps

_CACHE = {}
DEBUG = False


def _build_program():
    import concourse.bass as bass
    import concourse.bacc as bacc
    import concourse.tile as tile
    import concourse.mybir as mybir

    F32 = mybir.dt.float32
    F32R = mybir.dt.float32r
    BF16 = mybir.dt.bfloat16
    AF = mybir.ActivationFunctionType
    ALU = mybir.AluOpType
    AX = mybir.AxisListType
    I32 = mybir.dt.int32

    nc = bacc.Bacc("TRN2", target_bir_lowering=False, num_swdge_queues=4)

    # x typed f32r end-to-end: used only as the mm1 stationary operand
    x_d = nc.dram_tensor("x", [C, L], F32R, kind="ExternalInput")
    # xt pre-tiled on host to SBUF layout: xt[p, t*128+c] = x[c, t*128+p]
    xt_d = nc.dram_tensor("xt", [C, L], F32, kind="ExternalInput")
    xyh_d = nc.dram_tensor("xyh", [C, YW], F32, kind="ExternalInput")
    mrow_d = nc.dram_tensor("mrow", [1, HALF], F32R, kind="ExternalInput")
    out_d = nc.dram_tensor("out", [C, HALF], F32, kind="ExternalOutput")
    if DEBUG:
        dbg_d = {
            "d_norm2": nc.dram_tensor("d_norm2", [C, NLT], F32, kind="ExternalOutput"),
            "d_rs": nc.dram_tensor("d_rs", [C, NLT], F32, kind="ExternalOutput"),
            "d_yt": nc.dram_tensor("d_yt", [C, HALF], F32, kind="ExternalOutput"),
            "d_kn": nc.dram_tensor("d_kn", [C, L], F32, kind="ExternalOutput"),
            "d_sc0": nc.dram_tensor("d_sc0", [C, BLK], F32, kind="ExternalOutput"),
            "d_u0": nc.dram_tensor("d_u0", [C, BLK], F32, kind="ExternalOutput"),
            "d_sums0": nc.dram_tensor("d_sums0", [C, BLK], F32, kind="ExternalOutput"),
            "d_rec0": nc.dram_tensor("d_rec0", [C, BLK], F32, kind="ExternalOutput"),
        }

    with tile.TileContext(nc) as tc:
        with (
            tc.tile_pool(name="big", bufs=1) as big,
            tc.tile_pool(name="small", bufs=1) as small,
            tc.tile_pool(name="sqs", bufs=2) as sqs,
            tc.tile_pool(name="upool", bufs=8) as upool,
            tc.tile_pool(name="vpool", bufs=3) as vpool,
            tc.tile_pool(name="wpool", bufs=4) as wpool,
            tc.tile_pool(name="opool", bufs=4) as opool,
            tc.tile_pool(name="ps_sc", bufs=2, space=bass.MemorySpace.PSUM) as ps_sc,
            tc.tile_pool(name="ps_rec", bufs=1, space=bass.MemorySpace.PSUM) as ps_rec,
            tc.tile_pool(name="ps_sum", bufs=1, space=bass.MemorySpace.PSUM) as ps_sum,
        ):
            # ---- persistent SBUF tensors ----
            x_sb = big.tile([C, L], F32R, tag="x_sb")       # mm1 stationary
            xt_sb = big.tile([C, L], F32, tag="xt_sb")      # 32 tiles (128l, 128c)
            kn = big.tile([C, L], BF16, tag="kn")           # xt * rscale, l-major
            xyh_sb = big.tile([C, YW], F32, tag="xyh_sb")
            y1 = big.tile([C, YW], F32, tag="y1")
            y_t = big.tile([C, HALF], F32R, tag="y_t")
            mrow_sb = big.tile([1, HALF], F32R, tag="mrow_sb")
            ones_rf = big.tile([1, C], F32, tag="ones_rf")
            ones_row = big.tile([1, C], F32R, tag="ones_row")
            w_t = big.tile([C, HALF], F32, tag="w_t")       # (1-m)/9
            xm = big.tile([C, HALF], F32, tag="xm")         # x*m
            sqb_a = big.tile([C, 1280], F32, tag="sqb_a")   # xt*xt scratch (DVE)
            sqb_b = big.tile([C, 1280], F32, tag="sqb_b")   # xt*xt scratch (gp 1)
            sqb_c = big.tile([C, 768], F32, tag="sqb_c")    # xt*xt scratch (gp 2)
            ones_t = small.tile([C, C], BF16, tag="ones_t")
            ones_f = small.tile([C, C], F32, tag="ones_f")
            norm2 = small.tile([C, NLT], F32, tag="norm2")
            rs_a = small.tile([C, NLT], F32, tag="rs_a")
            rs_b = small.tile([C, NLT], F32, tag="rs_b")
            nt_a = small.tile([C, NLT], F32, tag="nt_a")
            eps_c = small.tile([C, 1], F32, tag="eps_c")
            shift_c = small.tile([C, 1], F32, tag="shift_c")

            # ---- input DMAs. The two HWDGE queues (sync, scalar) fair-share
            # ~100-150 GB/s each and are FIFO per queue, so wave 1 carries ONLY
            # what gates the first exps; everything else queues behind it.
            nc.scalar.dma_start(xt_sb[:, 0:768], xt_d[:, 0:768])
            nc.scalar.dma_start(x_sb[:, 0:512], x_d[:, 0:512])
            nc.scalar.dma_start(mrow_sb[:], mrow_d[:])
            nc.sync.dma_start(xyh_sb[:, 0:640], xyh_d[:, 0:640])
            nc.sync.dma_start(xyh_sb[:, 640:1216], xyh_d[:, 640:1216])
            # wave 2 (all on sync so the scalar/ACT engine is free for squares)
            nc.sync.dma_start(xt_sb[:, 768:2048], xt_d[:, 768:2048])
            nc.sync.dma_start(xyh_sb[:, 1216:YW], xyh_d[:, 1216:YW])
            nc.sync.dma_start(x_sb[:, 512:2048], x_d[:, 512:2048])
            # wave 3
            nc.sync.dma_start(xt_sb[:, 2048:3328], xt_d[:, 2048:3328])
            nc.sync.dma_start(x_sb[:, 2048:L], x_d[:, 2048:L])
            nc.sync.dma_start(xt_sb[:, 3328:L], xt_d[:, 3328:L])

            # ---- constants; exp table-set load paid during the DMA window
            nc.vector.memset(ones_f[:], 1.0)
            nc.vector.tensor_copy(ones_t[:], ones_f[:])
            nc.vector.memset(eps_c[:], 1e-7)
            nc.vector.memset(shift_c[:], -SHIFT)
            nc.vector.memset(ones_rf[:], 1.0)
            nc.vector.tensor_copy(ones_row[:], ones_rf[:])
            warm2 = small.tile([C, 1], F32, tag="warm2")
            nc.scalar.activation(warm2[:], eps_c[:], AF.Exp)

            # ---- norm2 chunk 0 (l-tiles 0..5) on ACT during the fill
            for lt in range(6):
                scr = sqs.tile([C, C], F32, tag="sq_scratch", name="scr")
                nc.scalar.activation(
                    scr[:], xt_sb[:, lt * C:(lt + 1) * C], AF.Square,
                    accum_out=norm2[:, lt:lt + 1],
                )

            # rsqrt via bit-trick seed + 3 Newton iterations (DVE only)
            def newton(l0, l1):
                cl = slice(l0, l1)
                nc.vector.tensor_scalar(nt_a[:, cl].bitcast(I32),
                                        norm2[:, cl].bitcast(I32), 1, None,
                                        op0=ALU.logical_shift_right)
                nc.vector.tensor_scalar(rs_b[:, cl].bitcast(I32),
                                        nt_a[:, cl].bitcast(I32),
                                        -1, 0x5f3759df,
                                        op0=ALU.mult, op1=ALU.add)
                src, dst = rs_b, rs_a
                for _ in range(3):
                    nc.vector.tensor_mul(nt_a[:, cl], src[:, cl], src[:, cl])
                    nc.vector.tensor_mul(nt_a[:, cl], nt_a[:, cl], norm2[:, cl])
                    nc.vector.tensor_scalar(nt_a[:, cl], nt_a[:, cl], -0.5, 1.5,
                                            op0=ALU.mult, op1=ALU.add)
                    nc.vector.tensor_mul(dst[:, cl], src[:, cl], nt_a[:, cl])
                    src, dst = dst, src
                # odd iteration count ends with the result in rs_a

            # norm2 chunks: A = l-tiles 6..16 squared on DVE (early deadline),
            # B = 16..26 and C = 26..32 squared on gpsimd (late deadlines)
            N2CH = [(6, 16), (16, 26), (26, 32)]
            N2BUF = {}

            def gp_sq(ci, buf):
                l0, l1 = N2CH[ci]
                w = (l1 - l0) * C
                N2BUF[ci] = buf
                nc.gpsimd.tensor_mul(buf[:, 0:w], xt_sb[:, l0 * C:l1 * C],
                                     xt_sb[:, l0 * C:l1 * C])

            def dve_sq(ci, buf):
                l0, l1 = N2CH[ci]
                w = (l1 - l0) * C
                N2BUF[ci] = buf
                nc.vector.tensor_mul(buf[:, 0:w], xt_sb[:, l0 * C:l1 * C],
                                     xt_sb[:, l0 * C:l1 * C])

            def dve_n2(ci):
                l0, l1 = N2CH[ci]
                n = l1 - l0
                v = N2BUF[ci][:, 0:n * C].rearrange("p (t c) -> p t c", c=C)
                nc.vector.tensor_reduce(norm2[:, l0:l1], v, AX.X, ALU.add)
                newton(l0, l1)

            def kn_chunk(l0, l1):
                for lt in range(l0, l1):
                    nc.vector.tensor_scalar_mul(
                        kn[:, lt * C:(lt + 1) * C], xt_sb[:, lt * C:(lt + 1) * C],
                        rs_a[:, lt:lt + 1])

            # ---- y = 3x3 box filter (row filter on xyh -> y1, then col filter)
            xv = xyh_sb[:].rearrange("p (r j) -> p r j", j=64)
            yv = y1[:].rearrange("p (r j) -> p r j", j=64)

            def y1_part(r0, r1):  # rows [r0, r1) of the 34-row halo image
                a, b = r0 * 64, r1 * 64
                a1 = max(a, 1)
                nc.vector.tensor_add(y1[:, a1:b], xyh_sb[:, a1 - 1:b - 1],
                                     xyh_sb[:, a1:b])
                nc.vector.tensor_add(y1[:, a1:b - 1], y1[:, a1:b - 1],
                                     xyh_sb[:, a1 + 1:b])
                nc.vector.tensor_add(yv[:, r0:r1, 0:1], xv[:, r0:r1, 0:1],
                                     xv[:, r0:r1, 1:2])
                nc.vector.tensor_add(yv[:, r0:r1, 63:64], xv[:, r0:r1, 62:63],
                                     xv[:, r0:r1, 63:64])

            def y_t_part(c0, c1):  # y_t columns [c0, c1), needs y1 rows thru (c1/64)+2
                nc.vector.tensor_add(y_t[:, c0:c1], y1[:, c0:c1],
                                     y1[:, c0 + 64:c1 + 64])
                nc.vector.tensor_add(y_t[:, c0:c1], y_t[:, c0:c1],
                                     y1[:, c0 + 128:c1 + 128])

            # --- fill-phase emission, critical-path ordered ---
            # DVE: y for block 0 first (gates mm1), then newton chunk 0 (gates
            # exp 0), then the rest interleaves with the stream.
            y1_part(0, 10)            # y1 rows 0..9  (xyh cols 0:640)
            y_t_part(0, 512)
            y1_part(10, 19)           # y1 rows 10..18 (xyh cols 640:1216)
            newton(0, 6)
            y_t_part(512, BLK)
            kn_chunk(0, 6)            # DVE: kn tiles for mm2 lt 0..5
            gp_sq(1, sqb_b)           # gpsimd: squares l-tiles 16..26

            # mask broadcast: mb = ones_row.T @ mrow (K=1 matmul) -> PSUM,
            # immediately converted to w_t = (1-m)/9 and xm = x*m so the
            # ps_sum buffer frees long before the first real sums tile
            def mask_half(half):
                t = ps_sum.tile([C, BLK], F32, tag="sums", name="mb")
                cs = half * BLK
                for h2 in range(2):
                    c2 = cs + h2 * 512
                    nc.tensor.matmul(
                        t[:, h2 * 512:(h2 + 1) * 512],
                        ones_row[:],
                        mrow_sb[:, c2:c2 + 512],
                        start=True, stop=True,
                    )
                nc.vector.tensor_scalar(w_t[:, cs:cs + BLK], t[:],
                                        -1.0 / 9.0, 1.0 / 9.0,
                                        op0=ALU.mult, op1=ALU.add)
                nc.vector.tensor_mul(xm[:, cs:cs + BLK],
                                     xyh_sb[:, 64 + cs:64 + cs + BLK], t[:])

            with tc.tile_wait_until(0.020):
                mask_half(0)

            # ---- main unified loop over (blk, lt) steps ----
            def mm1(blk, lt):
                sc = ps_sc.tile([C, BLK], F32, tag="sc", name="sc")
                for h2 in range(2):
                    cs = blk * BLK + h2 * 512
                    nc.tensor.matmul(
                        sc[:, h2 * 512:(h2 + 1) * 512],
                        x_sb[:, lt * C:(lt + 1) * C],
                        y_t[:, cs:cs + 512],
                        start=True, stop=True,
                    )
                return sc

            state = {}

            def emit_ones(blk, w, j, n_groups):
                sums = state[("sums", blk)]
                for h2 in range(2):
                    nc.tensor.matmul(
                        sums[:, h2 * 512:(h2 + 1) * 512],
                        ones_t[:],
                        w[:, h2 * 512:(h2 + 1) * 512],
                        start=(j == 0), stop=(j == n_groups - 1),
                    )

            def epilogue(blk, part):
                # part 0: R/Rm/ob for both 512-chunks (frees rec asap)
                # part 1: + xm and the output DMAs
                rec = state[("rec", blk)]
                sums = state[("sums", blk)]
                if part == 0:
                    if DEBUG and blk == 0:
                        dsm = big.tile([C, BLK], F32, tag="dsm", name="dsm")
                        nc.vector.tensor_copy(dsm[:], sums[:])
                        nc.sync.dma_start(dbg_d["d_sums0"][:], dsm[:])
                        drc = big.tile([C, BLK], F32, tag="drc", name="drc")
                        nc.vector.tensor_copy(drc[:], rec[:])
                        nc.sync.dma_start(dbg_d["d_rec0"][:], drc[:])
                    obs = []
                    for h2 in range(2):
                        cs = blk * BLK + h2 * 512
                        sl = slice(h2 * 512, (h2 + 1) * 512)
                        R = opool.tile([C, 512], F32, tag="R", name="R")
                        nc.vector.reciprocal_approx_fast(R[:], sums[:, sl])
                        Rm = opool.tile([C, 512], F32, tag="Rm", name="Rm")
                        nc.vector.tensor_mul(Rm[:], R[:], w_t[:, cs:cs + 512])
                        ob = opool.tile([C, 512], F32, tag=f"ob{h2}", name="ob")
                        nc.vector.tensor_mul(ob[:], rec[:, sl], Rm[:])
                        obs.append(ob)
                    state[("obs", blk)] = obs
                else:
                    obs = state.pop(("obs", blk))
                    for h2 in range(2):
                        cs = blk * BLK + h2 * 512
                        ob = obs[h2]
                        nc.vector.tensor_add(ob[:], ob[:], xm[:, cs:cs + 512])
                        nc.sync.dma_start(out_d[:, cs:cs + 512], ob[:])

            MM2_LAG = 5  # steps mm2 trails its exp in block 1's head
            pend_mm2 = []

            def mm2(blk, lt, u):
                rec = state[("rec", blk)]
                for h2 in range(2):
                    nc.tensor.matmul(
                        rec[:, h2 * 512:(h2 + 1) * 512],
                        kn[:, lt * C:(lt + 1) * C],
                        u[:, h2 * 512:(h2 + 1) * 512],
                        start=(lt == 0), stop=(lt == NLT - 1),
                    )

            # number of ones-mm accumulation groups per block:
            # 7 tree groups of 4 + last 4 l-tiles direct
            N_GROUPS = NLT // 4 - 1 + 4

            state[("sc", 0)] = mm1(0, 0)
            for step in range(NSTEP):
                blk, lt = divmod(step, NLT)
                if lt == 0:
                    state[("rec", blk)] = ps_rec.tile([C, BLK], F32, tag="rec", name="rec")
                    state[("sums", blk)] = ps_sum.tile([C, BLK], F32, tag="sums", name="sums")
                    state[("uq", blk)] = None      # u_prev for the tree
                    state[("vq", blk)] = None      # v_prev for the tree
                    state[("wq", blk)] = []        # lagged ones-mm queue
                    state[("wi", blk)] = 0

                # mm1 one step ahead (keeps ACT fed; emitted before mm2/ones)
                if step + 1 < NSTEP:
                    state[("sc", step + 1)] = mm1(*divmod(step + 1, NLT))

                # exp: u = Exp(sc * rscale[lt] - 20), bf16 out
                state_dbg_sc = state[("sc", step)]
                u = upool.tile([C, BLK], BF16, tag="u", name="u")
                nc.scalar.activation(u[:], state.pop(("sc", step))[:], AF.Exp,
                                     bias=shift_c[:],
                                     scale=rs_a[:, lt:lt + 1])

                if DEBUG and step == 0:
                    dsc = big.tile([C, BLK], F32, tag="dsc", name="dsc")
                    nc.vector.tensor_copy(dsc[:], state_dbg_sc[:])
                    nc.sync.dma_start(dbg_d["d_sc0"][:], dsc[:])
                    du = big.tile([C, BLK], F32, tag="du", name="du")
                    nc.vector.tensor_copy(du[:], u[:])
                    nc.sync.dma_start(dbg_d["d_u0"][:], du[:])

                # mm2 lagged: block 0's head waits for the kn chunk emitted
                # in the same step's extras; block 1's head waits for the
                # block-0 epilogue to free the rec PSUM buffer
                pend_mm2.append((blk, lt, u))
                if blk == 0:
                    lag = 2 if lt < 8 else 0
                else:
                    lag = max(0, min(MM2_LAG, 12 - lt))
                while len(pend_mm2) > lag:
                    mm2(*pend_mm2.pop(0))

                # column sums: 2-level pairwise bf16 tree on DVE; PE finishes
                # with a lagged ones-mm per group of 4. Last 4 l-tiles: direct
                # PE ones-mm (shortens the tail before the epilogue).
                if lt >= NLT - 4:
                    j = state[("wi", blk)]
                    emit_ones(blk, u, j, N_GROUPS)
                    state[("wi", blk)] = j + 1
                elif lt % 2 == 0:
                    state[("uq", blk)] = u
                else:
                    v = vpool.tile([C, BLK], BF16, tag="v", name="v")
                    nc.vector.tensor_add(v[:], state[("uq", blk)][:], u[:])
                    if state[("vq", blk)] is None:
                        state[("vq", blk)] = v
                    else:
                        w = wpool.tile([C, BLK], BF16, tag="w", name="w")
                        nc.vector.tensor_add(w[:], state[("vq", blk)][:], v[:])
                        state[("vq", blk)] = None
                        state[("wq", blk)].append((w, state[("wi", blk)]))
                        state[("wi", blk)] = state[("wi", blk)] + 1
                        if len(state[("wq", blk)]) > 2:
                            emit_ones(blk, *state[("wq", blk)].pop(0),
                                      N_GROUPS)
                if lt == NLT - 5:
                    for w, j in state[("wq", blk)]:
                        emit_ones(blk, w, j, N_GROUPS)
                    state[("wq", blk)] = []

                # interleaved prologue/epilogue extras (block 0 stream feeds
                # later chunks; block 1 stream drains block 0's epilogue)
                if blk == 0:
                    if lt == 1:
                        dve_sq(0, sqb_a)      # squares l-tiles 6..15 on DVE
                        dve_n2(0)
                    elif lt == 2:
                        with tc.tile_wait_until(0.025):
                            mask_half(1)
                    elif lt == 3:
                        kn_chunk(6, 12)
                        with tc.tile_wait_until(0.030):
                            y1_part(19, 34)   # y1 rows 19..33 (xyh part B)
                    elif lt == 4:
                        with tc.tile_wait_until(0.032):
                            y_t_part(BLK, HALF)   # y_t block 1
                    elif lt == 5:
                        kn_chunk(12, 16)
                        gp_sq(2, sqb_c)       # gpsimd: squares l-tiles 26..31
                    elif lt == 7:
                        dve_n2(1)             # reduce+newton l-tiles 16..25
                    elif lt == 8:
                        kn_chunk(16, 22)
                    elif lt == 11:
                        dve_n2(2)             # reduce+newton l-tiles 26..31
                    elif lt == 12:
                        kn_chunk(22, 26)
                    elif lt == 14:
                        kn_chunk(26, NLT)
                else:
                    if lt == 1:
                        epilogue(0, 0)
                    elif lt == 3:
                        epilogue(0, 1)

            # drain: block 1 epilogue
            epilogue(1, 0)
            epilogue(1, 1)
            if DEBUG:
                nc.sync.dma_start(dbg_d["d_norm2"][:], norm2[:])
                nc.sync.dma_start(dbg_d["d_rs"][:], rs_a[:])
                dyt = big.tile([C, HALF], F32, tag="dyt", name="dyt")
                nc.vector.tensor_copy(dyt[:], y_t[:])
                nc.sync.dma_start(dbg_d["d_yt"][:], dyt[:])
                dkn = big.tile([C, L], F32, tag="dkn", name="dkn")
                nc.vector.tensor_copy(dkn[:], kn[:])
                nc.sync.dma_start(dbg_d["d_kn"][:], dkn[:])

    nc.finalize()
    return nc


def _get_program():
    if "nc" not in _CACHE:
        _CACHE["nc"] = _build_program()
    return _CACHE["nc"]


def _make_in_maps(fg, mk):
    in_maps = []
    for core in range(8):
        b, h = core // 2, core % 2
        x = np.ascontiguousarray(fg[b].reshape(C, L))
        xt = np.ascontiguousarray(
            x.reshape(C, L // C, C).transpose(2, 1, 0).reshape(C, L))
        xi = fg[b].reshape(C, 64, 64)
        rows = np.zeros((C, 34, 64), np.float32)
        r0 = 32 * h - 1
        lo, hi = max(0, r0), min(64, r0 + 34)
        rows[:, lo - r0:hi - r0, :] = xi[:, lo:hi, :]
        xyh = np.ascontiguousarray(rows.reshape(C, YW))
        mrow = np.ascontiguousarray(
            mk[b].reshape(1, L)[:, h * HALF:(h + 1) * HALF])
        in_maps.append({"x": x, "xt": xt, "xyh": xyh, "mrow": mrow})
    return in_maps


def kernel(foreground, mask):
    fg = np.ascontiguousarray(np.asarray(foreground, dtype=np.float32))
    mk = np.ascontiguousarray(np.asarray(mask, dtype=np.float32))
    nc = _get_program()
    in_maps = _make_in_maps(fg, mk)

    from concourse.bass_utils import run_bass_kernel_spmd
    res = run_bass_kernel_spmd(nc, in_maps, core_ids=list(range(8)))

    out = np.empty((4, C, L), np.float32)
    for core in range(8):
        b, h = core // 2, core % 2
        out[b][:, h * HALF:(h + 1) * HALF] = res.results[core]["out"]
    if DEBUG:
        _CACHE["debug"] = [dict(r) for r in res.results]
    return out.reshape(4, C, 64, 64)
